# revision 36
# baseline (speedup 1.0000x reference)
"""BertSelfAttention forward on 8 Trainium2 NeuronCores (Bass/Tile).

Problem: B=2, S=2048, HIDDEN=1024, 16 heads x head_dim 64, fp32 I/O.

Sharding: core c handles batch b = c//4 and head-group g = c%4
(heads 4g..4g+4 == hidden columns 256g..256g+256). Attention is
embarrassingly parallel per (batch, head): no collectives; each core
computes a disjoint [S, 256] slice of the output.

Per-core device program (matmuls bf16, fp32 PSUM accumulate):
  1. Load hs fp32 in row-batches, cast to bf16 on DVE, transpose on PE
     into per-(column-chunk, row-group) hsT tiles.
  2. qT/kT/vT [256d, 2048s] = W.T @ hsT, W chunk stationary. Biases
     fused into the PSUM->SBUF copies as per-partition DVE scalar-adds.
     v transposed back to natural [s, d] on the PE and stored with a
     constant-1.0 65th column (softmax denominator trick).
  3. Scores transposed [k, q]: two heads packed into PE rows 0-63 /
     64-127 (row tiling); per key tile the kT slice is streamed against
     two 512-wide q-chunks into one [128, 1024] psum pair. exp on
     ScalarE straight from PSUM with scale=1/8; the additive attention
     mask folds into the per-partition bias (exact reproduction of
     reference masking; all-ones mask -> 0). No max-subtraction: scores
     ~ N(0,1) by construction, exp is safe in fp32 and softmax is
     shift-invariant.
  4. ctxT[65, q] = [v | 1].T @ probsT, v-slice stationary, probs
     streaming at N=512. Row 64 = softmax denominator.
  5. Copy ctxT to SBUF, PE-transpose back to natural, reciprocal +
     per-partition scalar-mul on DVE, DMA out.

ScalarE's exp stream (~140us) is the bottleneck engine, so the emission
is built around keeping it saturated: all projection and ctx work is
chopped into ~2-3us pieces on a global work queue that the scores/exp
streams drain between key tiles, so the in-order PE queue always has
off-critical-path work without ever delaying the next psum refill.
A short dependency-chained warm-up matmul chain keeps the PE's HAM
clock-gate at full rate through the initial DMA window.
"""

import sys
from collections import deque
from contextlib import ExitStack

for _p in ("/opt/trn_rl_repo",):
    if _p not in sys.path:
        sys.path.insert(0, _p)

import numpy as np

import concourse.bass as bass  # noqa: F401
import concourse.mybir as mybir
import concourse.tile as tile
from concourse import bacc
from concourse.bass_utils import run_bass_kernel_spmd
from concourse.masks import make_identity

B, S, HID = 2, 2048, 1024
NH, HD = 16, 64
N_CORES = 8
GH = 4  # heads per core
GD = GH * HD  # 256
P = 128
ST = S // P  # 16 seq tiles
HC = HID // P  # 8 hidden chunks
QC = 4  # q chunks of 512
QW = S // QC  # 512
F32 = mybir.dt.float32
BF16 = mybir.dt.bfloat16
EXP = mybir.ActivationFunctionType.Exp

_CACHE = {}


def _build_nc(plain_mask: bool):
    nc = bacc.Bacc("TRN2", target_bir_lowering=False, debug=False, num_devices=N_CORES)

    hs = nc.dram_tensor("hs", [S, HID], F32, kind="ExternalInput").ap()
    w = nc.dram_tensor("w", [HID, 3 * GD], F32, kind="ExternalInput").ap()
    # packed per-partition smalls: cols 0-1 bq, 2-3 bk, 4-5 bv, 6-21 mask
    small_t = nc.dram_tensor("small_t", [P, 22], F32, kind="ExternalInput").ap()
    hs16d = nc.dram_tensor("hs16d", [512, HID], BF16).ap()
    warm_sink = nc.dram_tensor("warm_sink", [P, 1], F32).ap()
    y = nc.dram_tensor("y", [S, GD], F32, kind="ExternalOutput").ap()

    with tile.TileContext(nc) as tc:
        with (
            tc.tile_pool(name="const", bufs=1) as constp,
            tc.tile_pool(name="big", bufs=1) as bigp,
            tc.tile_pool(name="outp", bufs=4) as outp,
            tc.tile_pool(name="misc", bufs=4) as miscp,
            tc.tile_pool(name="probs", bufs=1) as probsp,
            tc.tile_pool(name="ctxp", bufs=1) as ctxp,
            tc.tile_pool(name="psS", bufs=1, space="PSUM") as psS,
            tc.tile_pool(name="psT", bufs=1, space="PSUM") as psT,
        ):
            # ---- constants (DMA order matters: hs batches go first on the
            # HWDGE FIFO; W rides the same FIFO behind the critical ones) ----
            id16 = constp.tile([P, P], BF16)
            make_identity(nc, id16[:])
            id32 = constp.tile([P, P], F32)
            make_identity(nc, id32[:])
            w_sb = constp.tile([P, HC, 3 * GD], BF16)

            hsTt = [
                [bigp.tile([P, QW], BF16, name=f"hsT{hc}_{stg}") for stg in range(QC)]
                for hc in range(HC)
            ]
            qTc = [[None] * QC for _ in range(2)]
            kTc = [[None] * QC for _ in range(2)]
            for dc in range(2):
                for sc in range(QC):
                    qTc[dc][sc] = bigp.tile([P, QW], BF16, name=f"qT{dc}_{sc}")
                    kTc[dc][sc] = bigp.tile([P, QW], BF16, name=f"kT{dc}_{sc}")
            v_sb = bigp.tile([P, ST, GH, HD + 1], BF16)
            nc.vector.memset(v_sb[:], 1.0)  # col 64 stays 1.0 (denominator)

            # ---- phase 1 DMA choreography: two DGE queues in parallel,
            # most-critical transfers first in each FIFO.
            #   sync FIFO:   small, g0, g2, Wq, g4, Wv, stg3 transpose-loads
            #   gpsimd FIFO: g1, g3, Wk, g5, casts for stg3 (DRAM->DRAM)
            small_sb = constp.tile([P, 22], F32)
            nc.sync.dma_start(small_sb[:], small_t[:])
            bq_sb, bk_sb, bv_sb = small_sb[:, 0:2], small_sb[:, 2:4], small_sb[:, 4:6]
            mask_sb = small_sb[:, 6:22]

            def load_hs(g, eng):
                hsf = bigp.tile(
                    [P, 2, HID], F32, tag="hsf", bufs=3, name=f"hsf{g}"
                )
                eng.dma_start(
                    hsf[:],
                    hs[2 * g * P : 2 * (g + 1) * P, :].rearrange(
                        "(j p) h -> p j h", p=P
                    ),
                )
                h16 = bigp.tile(
                    [P, 2, HID], BF16, tag="hs16", bufs=4, name=f"hs16_{g}"
                )
                nc.vector.tensor_copy(h16[:], hsf[:])
                return h16

            def load_w(wi):
                wf = bigp.tile(
                    [P, HC, GD], F32, tag="hsf", bufs=3, name=f"wf{wi}"
                )
                nc.sync.dma_start(
                    wf[:],
                    w[:, wi * GD : (wi + 1) * GD].rearrange(
                        "(c p) d -> p c d", p=P
                    ),
                )
                nc.vector.tensor_copy(
                    w_sb[:, :, wi * GD : (wi + 1) * GD], wf[:]
                )

            hs16 = [None] * 6
            hs16[0] = load_hs(0, nc.sync)
            hs16[1] = load_hs(1, nc.gpsimd)
            hs16[2] = load_hs(2, nc.sync)
            hs16[3] = load_hs(3, nc.gpsimd)
            load_w(0)  # Wq on sync
            # Wk on gpsimd (fp32 plain, DVE cast via hsf-tag staging)
            wfk = bigp.tile([P, HC, GD], F32, tag="hsf", bufs=3, name="wfk")
            nc.gpsimd.dma_start(
                wfk[:], w[:, GD : 2 * GD].rearrange("(c p) d -> p c d", p=P)
            )
            nc.vector.tensor_copy(w_sb[:, :, GD : 2 * GD], wfk[:])
            hs16[4] = load_hs(4, nc.sync)
            hs16[5] = load_hs(5, nc.gpsimd)
            load_w(2)  # Wv on sync
            # stg3 of hsT comes via DRAM-side cast + xbar transpose loads
            nc.gpsimd.dma_start(hs16d[0:256, :], hs[1536:1792, :])
            nc.gpsimd.dma_start(hs16d[256:512, :], hs[1792:2048, :])
            for hc in range(HC):
                nc.sync.dma_start(
                    hsTt[hc][3][:], hs16d[:, hc * P : (hc + 1) * P], transpose=True
                )

            # ---- PE warm-up chain: keeps HAM at full clock through the
            # initial DMA window so the transposes/projections run at 2.4GHz
            psQ_stack = ExitStack()
            psQ = psQ_stack.enter_context(
                tc.tile_pool(name="psQ", bufs=1, space="PSUM")
            )
            dums = constp.tile([P, 512], BF16)
            nc.vector.memset(dums[:], 0.25)
            wmp = psQ.tile([P, QW], F32, tag="ps", bufs=2, name="wmp")
            for i in range(44):
                nc.tensor.matmul(
                    wmp[:], lhsT=dums[:, 0:P], rhs=dums[:], start=(i == 0),
                    stop=(i == 43),
                )
            wsb = miscp.tile([P, 1], F32, tag="wsb")
            nc.vector.tensor_copy(wsb[:], wmp[:, 0:1])
            nc.sync.dma_start(warm_sink[:], wsb[:])

            def hs_transpose(stg, hcs):
                for hc in hcs:
                    pt = psT.tile([P, 512], BF16, tag="t", bufs=2)
                    for j in range(4):
                        g, jj = divmod(stg * 4 + j, 2)
                        nc.tensor.transpose(
                            pt[:, j * P : (j + 1) * P],
                            hs16[g][:, jj, hc * P : (hc + 1) * P],
                            id16[:],
                        )
                    nc.vector.tensor_copy(hsTt[hc][stg][:], pt[:])

            for stg in range(2):
                hs_transpose(stg, range(HC))

            # ---- work queue machinery ----
            work = deque()

            def pump(n=None):
                if n is None:
                    n = 2 if len(work) > 22 else 1
                for _ in range(n):
                    if not work:
                        return
                    work.popleft()()

            proj_state = {}
            vt_tiles = {}
            psC_holder = {}

            def proj_quarter(dst_chunks, b_sb, w_off, dc, scg, q):
                scs = (2 * scg, 2 * scg + 1)
                key = (w_off, dc, scg)
                if q == 0:
                    proj_state[key] = [
                        psQ.tile([P, QW], F32, tag="ps", bufs=2, name=f"pp{i}")
                        for i in range(2)
                    ]
                pps = proj_state[key]
                for hc in range(2 * q, 2 * q + 2):
                    for i, sc in enumerate(scs):
                        nc.tensor.matmul(
                            pps[i][:],
                            lhsT=w_sb[:, hc, w_off + dc * P : w_off + (dc + 1) * P],
                            rhs=hsTt[hc][sc][:],
                            start=(hc == 0),
                            stop=(hc == HC - 1),
                        )
                if q == 3:
                    for i, sc in enumerate(scs):
                        nc.vector.tensor_scalar_add(
                            out=dst_chunks[sc][:],
                            in0=pps[i][:],
                            scalar1=b_sb[:, dc : dc + 1],
                        )
                    del proj_state[key]

            def proj_group(dst_chunks, b_sb, w_off, dc, scg):
                for q in range(4):
                    proj_quarter(dst_chunks, b_sb, w_off, dc, scg, q)

            def v_dst(dc):
                if dc not in vt_tiles:
                    vt_tiles[dc] = ctxp.tile(
                        [P, S], BF16, tag=f"vt{dc}", bufs=1, name=f"vt{dc}"
                    )
                vt = vt_tiles[dc]
                return [vt[:, sc * QW : (sc + 1) * QW] for sc in range(QC)]

            def v_back(dc, stg):
                vt = vt_tiles[dc]
                pt = psT.tile([P, 512], BF16, tag="t", bufs=2)
                for j in range(4):
                    st = stg * 4 + j
                    nc.tensor.transpose(
                        pt[:, j * P : (j + 1) * P],
                        vt[:, st * P : (st + 1) * P],
                        id16[:],
                    )
                nc.vector.tensor_copy(
                    v_sb[:, stg * 4 : (stg + 1) * 4, 2 * dc : 2 * dc + 2, 0:HD],
                    pt[:].rearrange("p (a h d) -> p a h d", h=2, d=HD),
                )

            def pool_switch():
                psQ_stack.close()
                psC_holder["pool"] = tc.alloc_tile_pool(
                    name="psC", bufs=1, space="PSUM"
                )

            # ---- attention emitters ----
            def scores_emit(pair, qcg):
                pts = {0: [], 1: []}
                q0, q1 = 2 * qcg, 2 * qcg + 1
                for kt in range(ST):
                    sc, kk = divmod(kt, 4)
                    for hh, rows, tp in (
                        (0, slice(0, 64), (0, 0)),
                        (1, slice(64, 128), (64, 0)),
                    ):
                        sps = psS.tile([P, 2 * QW], F32, tag=f"s{hh}", bufs=1)
                        for j, qq in ((0, q0), (1, q1)):
                            nc.tensor.matmul(
                                sps[:, j * QW : (j + 1) * QW],
                                lhsT=kTc[pair][sc][rows, kk * P : (kk + 1) * P],
                                rhs=qTc[pair][qq][rows, :],
                                start=True,
                                stop=True,
                                tile_position=tp,
                            )
                        pt = probsp.tile(
                            [P, 2, QW], BF16, tag=f"p{hh}", bufs=17,
                            name=f"pt{hh}_{kt}",
                        )
                        if plain_mask:
                            nc.scalar.activation(
                                pt[:],
                                sps[:].rearrange("p (a b) -> p a b", b=QW),
                                EXP,
                                scale=0.125,
                            )
                        else:
                            nc.scalar.activation(
                                pt[:],
                                sps[:].rearrange("p (a b) -> p a b", b=QW),
                                EXP,
                                bias=mask_sb[:, kt : kt + 1],
                                scale=0.125,
                            )
                        pts[hh].append(pt)
                    pump()
                return pts

            def ctx_pieces(pair, qcg, pts):
                pieces = []
                for hh in range(2):
                    h = 2 * pair + hh
                    pcs = [None, None]

                    def make_accum(kq, hh=hh, h=h, pcs=pcs):
                        def accum():
                            if kq == 0:
                                psC = psC_holder["pool"]
                                for j in range(2):
                                    pcs[j] = psC.tile(
                                        [P, QW], F32, tag="ca", bufs=2,
                                        name=f"pc{hh}{j}",
                                    )
                            for kt in range(4 * kq, 4 * kq + 4):
                                for j in range(2):
                                    nc.tensor.matmul(
                                        pcs[j][0 : HD + 1, :],
                                        lhsT=v_sb[:, kt, h, :],
                                        rhs=pts[hh][kt][:, j],
                                        start=(kt == 0),
                                        stop=(kt == ST - 1),
                                        skip_group_check=True,
                                    )

                        return accum

                    for kq in range(4):
                        pieces.append(make_accum(kq))

                    def make_post(j, hh=hh, h=h, pcs=pcs):
                        def post():
                            qq = 2 * qcg + j
                            ctxs = ctxp.tile([P, QW], F32, tag="ctxs", bufs=2)
                            nc.vector.tensor_copy(
                                ctxs[0 : HD + 1, :], pcs[j][0 : HD + 1, :]
                            )
                            pd = psT.tile([P, QC * (HD + 1)], F32, tag="t", bufs=2)
                            pdv = pd[:].rearrange("p (q e) -> p q e", e=HD + 1)
                            for qt in range(QC):
                                nc.tensor.transpose(
                                    pdv[:, qt],
                                    ctxs[0 : HD + 1, qt * P : (qt + 1) * P],
                                    id32[0 : HD + 1, 0 : HD + 1],
                                )
                            rec = miscp.tile([P, QC], F32, tag="rec")
                            nc.vector.reciprocal(rec[:], pdv[:, :, HD])
                            ot = outp.tile([P, QC, HD], F32, tag="ot")
                            for qt in range(QC):
                                nc.vector.tensor_scalar_mul(
                                    out=ot[:, qt],
                                    in0=pdv[:, qt, 0:HD],
                                    scalar1=rec[:, qt : qt + 1],
                                )
                            nc.sync.dma_start(
                                y[qq * QW : (qq + 1) * QW, h * HD : (h + 1) * HD]
                                .rearrange("(q p) d -> p q d", p=P),
                                ot[:],
                            )

                        return post

                    pieces.append(make_post(0))
                    pieces.append(make_post(1))
                return pieces

            # ---- emission ----
            # critical path first: q/k chunks covering the first key tiles
            proj_group(qTc[0], bq_sb, 0, 0, 0)
            proj_group(kTc[0], bk_sb, GD, 0, 0)
            # everything else rides the work queue (rest of the hs
            # transposes first -- the kT chunks for key tiles 8..15 need
            # them and must land within the first scores stream)
            work.append(lambda: hs_transpose(2, range(0, 4)))
            work.append(lambda: hs_transpose(2, range(4, 8)))
            for args in (
                (kTc[0], bk_sb, GD, 0, 1),
                (qTc[0], bq_sb, 0, 0, 1),
                (qTc[1], bq_sb, 0, 1, 0),
                (qTc[1], bq_sb, 0, 1, 1),
                (kTc[1], bk_sb, GD, 1, 0),
                (kTc[1], bk_sb, GD, 1, 1),
            ):
                for q in range(4):
                    work.append(lambda a=args, q=q: proj_quarter(*a, q))
            for dc in range(2):
                for scg in range(2):
                    for q in range(4):
                        work.append(
                            lambda dc=dc, scg=scg, q=q: proj_quarter(
                                v_dst(dc), bv_sb, 2 * GD, dc, scg, q
                            )
                        )
            for dc in range(2):
                for stg in range(QC):
                    work.append(lambda dc=dc, stg=stg: v_back(dc, stg))
            work.append(pool_switch)

            pts = scores_emit(0, 0)
            prev = (0, 0, pts)
            for pair, qcg in ((0, 1), (1, 0), (1, 1)):
                work.extend(ctx_pieces(prev[0], prev[1], prev[2]))
                pts = scores_emit(pair, qcg)
                prev = (pair, qcg, pts)
            while work:
                pump(4)
            for fn in ctx_pieces(prev[0], prev[1], prev[2]):
                fn()
            if "pool" in psC_holder:
                psC_holder["pool"].release()
    nc.compile()
    return nc


def _make_in_maps(hidden_states, attention_mask, Wq, bq, Wk, bk, Wv, bv):
    min_val = np.finfo(np.float32).min
    in_maps = []
    for c in range(N_CORES):
        b, g = divmod(c, N_CORES // B)
        sl = slice(GD * g, GD * (g + 1))
        small = np.concatenate(
            [
                bq[sl].reshape(2, P).T,
                bk[sl].reshape(2, P).T,
                bv[sl].reshape(2, P).T,
                ((1.0 - attention_mask[b]) * min_val)
                .astype(np.float32)
                .reshape(ST, P)
                .T,
            ],
            axis=1,
        ).astype(np.float32)
        in_maps.append(
            {
                "hs": np.ascontiguousarray(hidden_states[b]),
                "w": np.ascontiguousarray(
                    np.concatenate([Wq[:, sl], Wk[:, sl], Wv[:, sl]], axis=1)
                ),
                "small_t": np.ascontiguousarray(small),
            }
        )
    return in_maps


def kernel(hidden_states, attention_mask, Wq, bq, Wk, bk, Wv, bv):
    hidden_states = np.asarray(hidden_states, dtype=np.float32)
    attention_mask = np.asarray(attention_mask, dtype=np.float32)
    Wq, Wk, Wv = (np.asarray(a, dtype=np.float32) for a in (Wq, Wk, Wv))
    bq, bk, bv = (np.asarray(a, dtype=np.float32) for a in (bq, bk, bv))

    plain = bool(np.all(attention_mask == 1.0))
    key = ("nc", plain)
    if key not in _CACHE:
        _CACHE[key] = _build_nc(plain)
    nc = _CACHE[key]
    _CACHE["nc"] = nc  # most-recent, for test harness reuse

    in_maps = _make_in_maps(hidden_states, attention_mask, Wq, bq, Wk, bk, Wv, bv)
    res = run_bass_kernel_spmd(nc, in_maps, list(range(N_CORES)))
    out = np.empty((B, S, HID), dtype=np.float32)
    for c in range(N_CORES):
        b, g = divmod(c, N_CORES // B)
        out[b, :, GD * g : GD * (g + 1)] = res.results[c]["y"]
    return out


# revision 37
# speedup vs baseline: 1.0455x; 1.0455x over previous
"""BertSelfAttention forward on 8 Trainium2 NeuronCores (Bass/Tile).

Problem: B=2, S=2048, HIDDEN=1024, 16 heads x head_dim 64, fp32 I/O.

Sharding: core c handles batch b = c//4 and head-group g = c%4
(heads 4g..4g+4 == hidden columns 256g..256g+256). Attention is
embarrassingly parallel per (batch, head): no collectives; each core
computes a disjoint [S, 256] slice of the output.

Per-core device program (matmuls bf16, fp32 PSUM accumulate):
  1. Load hs fp32 in row-batches, cast to bf16 on DVE, transpose on PE
     into per-(column-chunk, row-group) hsT tiles.
  2. qT/kT/vT [256d, 2048s] = W.T @ hsT, W chunk stationary. Biases
     fused into the PSUM->SBUF copies as per-partition DVE scalar-adds.
     v transposed back to natural [s, d] on the PE and stored with a
     constant-1.0 65th column (softmax denominator trick).
  3. Scores transposed [k, q]: two heads packed into PE rows 0-63 /
     64-127 (row tiling); per key tile the kT slice is streamed against
     two 512-wide q-chunks into one [128, 1024] psum pair. exp on
     ScalarE straight from PSUM with scale=1/8; the additive attention
     mask folds into the per-partition bias (exact reproduction of
     reference masking; all-ones mask -> 0). No max-subtraction: scores
     ~ N(0,1) by construction, exp is safe in fp32 and softmax is
     shift-invariant.
  4. ctxT[65, q] = [v | 1].T @ probsT, v-slice stationary, probs
     streaming at N=512. Row 64 = softmax denominator.
  5. Copy ctxT to SBUF, PE-transpose back to natural, reciprocal +
     per-partition scalar-mul on DVE, DMA out.

ScalarE's exp stream (~140us) is the bottleneck engine, so the emission
is built around keeping it saturated: all projection and ctx work is
chopped into ~2-3us pieces on a global work queue that the scores/exp
streams drain between key tiles, so the in-order PE queue always has
off-critical-path work without ever delaying the next psum refill.
A short dependency-chained warm-up matmul chain keeps the PE's HAM
clock-gate at full rate through the initial DMA window.
"""

import sys
from collections import deque
from contextlib import ExitStack

for _p in ("/opt/trn_rl_repo",):
    if _p not in sys.path:
        sys.path.insert(0, _p)

import numpy as np

import concourse.bass as bass  # noqa: F401
import concourse.mybir as mybir
import concourse.tile as tile
from concourse import bacc
from concourse.bass_utils import run_bass_kernel_spmd
from concourse.masks import make_identity

B, S, HID = 2, 2048, 1024
NH, HD = 16, 64
N_CORES = 8
GH = 4  # heads per core
GD = GH * HD  # 256
P = 128
ST = S // P  # 16 seq tiles
HC = HID // P  # 8 hidden chunks
QC = 4  # q chunks of 512
QW = S // QC  # 512
F32 = mybir.dt.float32
BF16 = mybir.dt.bfloat16
EXP = mybir.ActivationFunctionType.Exp

_CACHE = {}


def _build_nc(plain_mask: bool):
    nc = bacc.Bacc("TRN2", target_bir_lowering=False, debug=False, num_devices=N_CORES)

    hs = nc.dram_tensor("hs", [S, HID], F32, kind="ExternalInput").ap()
    w = nc.dram_tensor("w", [HID, 3 * GD], F32, kind="ExternalInput").ap()
    # packed per-partition smalls: cols 0-1 bq, 2-3 bk, 4-5 bv, 6-21 mask
    small_t = nc.dram_tensor("small_t", [P, 22], F32, kind="ExternalInput").ap()
    hs16d = nc.dram_tensor("hs16d", [512, HID], BF16).ap()
    warm_sink = nc.dram_tensor("warm_sink", [P, 1], F32).ap()
    y = nc.dram_tensor("y", [S, GD], F32, kind="ExternalOutput").ap()

    with tile.TileContext(nc) as tc:
        with (
            tc.tile_pool(name="const", bufs=1) as constp,
            tc.tile_pool(name="big", bufs=1) as bigp,
            tc.tile_pool(name="outp", bufs=4) as outp,
            tc.tile_pool(name="misc", bufs=4) as miscp,
            tc.tile_pool(name="probs", bufs=1) as probsp,
            tc.tile_pool(name="ctxp", bufs=1) as ctxp,
            tc.tile_pool(name="psS", bufs=1, space="PSUM") as psS,
            tc.tile_pool(name="psT", bufs=1, space="PSUM") as psT,
        ):
            # ---- constants (DMA order matters: hs batches go first on the
            # HWDGE FIFO; W rides the same FIFO behind the critical ones) ----
            id16 = constp.tile([P, P], BF16)
            make_identity(nc, id16[:])
            id32 = constp.tile([P, P], F32)
            make_identity(nc, id32[:])
            w_sb = constp.tile([P, HC, 3 * GD], BF16)

            hsTt = [
                [bigp.tile([P, QW], BF16, name=f"hsT{hc}_{stg}") for stg in range(QC)]
                for hc in range(HC)
            ]
            qTc = [[None] * QC for _ in range(2)]
            kTc = [[None] * QC for _ in range(2)]
            for dc in range(2):
                for sc in range(QC):
                    qTc[dc][sc] = bigp.tile([P, QW], BF16, name=f"qT{dc}_{sc}")
                    kTc[dc][sc] = bigp.tile([P, QW], BF16, name=f"kT{dc}_{sc}")
            v_sb = bigp.tile([P, ST, GH, HD + 1], BF16)
            nc.vector.memset(v_sb[:], 1.0)  # col 64 stays 1.0 (denominator)

            # ---- phase 1 DMA choreography: two DGE queues in parallel,
            # most-critical transfers first in each FIFO.
            #   sync FIFO:   small, g0, g2, Wq, g4, Wv, stg3 transpose-loads
            #   gpsimd FIFO: g1, g3, Wk, g5, casts for stg3 (DRAM->DRAM)
            small_sb = constp.tile([P, 22], F32)
            nc.sync.dma_start(small_sb[:], small_t[:])
            bq_sb, bk_sb, bv_sb = small_sb[:, 0:2], small_sb[:, 2:4], small_sb[:, 4:6]
            mask_sb = small_sb[:, 6:22]

            def load_hs(g, eng):
                hsf = bigp.tile(
                    [P, 2, HID], F32, tag="hsf", bufs=3, name=f"hsf{g}"
                )
                eng.dma_start(
                    hsf[:],
                    hs[2 * g * P : 2 * (g + 1) * P, :].rearrange(
                        "(j p) h -> p j h", p=P
                    ),
                )
                h16 = bigp.tile(
                    [P, 2, HID], BF16, tag="hs16", bufs=4, name=f"hs16_{g}"
                )
                nc.vector.tensor_copy(h16[:], hsf[:])
                return h16

            def load_w(wi):
                wf = bigp.tile(
                    [P, HC, GD], F32, tag="hsf", bufs=3, name=f"wf{wi}"
                )
                nc.sync.dma_start(
                    wf[:],
                    w[:, wi * GD : (wi + 1) * GD].rearrange(
                        "(c p) d -> p c d", p=P
                    ),
                )
                nc.vector.tensor_copy(
                    w_sb[:, :, wi * GD : (wi + 1) * GD], wf[:]
                )

            hs16 = [None] * 6
            hs16[0] = load_hs(0, nc.sync)
            hs16[1] = load_hs(1, nc.gpsimd)
            hs16[2] = load_hs(2, nc.sync)
            hs16[3] = load_hs(3, nc.gpsimd)
            load_w(0)  # Wq on sync
            # Wk on gpsimd (fp32 plain, DVE cast via hsf-tag staging)
            wfk = bigp.tile([P, HC, GD], F32, tag="hsf", bufs=3, name="wfk")
            nc.gpsimd.dma_start(
                wfk[:], w[:, GD : 2 * GD].rearrange("(c p) d -> p c d", p=P)
            )
            nc.vector.tensor_copy(w_sb[:, :, GD : 2 * GD], wfk[:])
            hs16[4] = load_hs(4, nc.sync)
            hs16[5] = load_hs(5, nc.gpsimd)
            load_w(2)  # Wv on sync
            # stg3 of hsT comes via DRAM-side cast + xbar transpose loads
            nc.gpsimd.dma_start(hs16d[0:256, :], hs[1536:1792, :])
            nc.gpsimd.dma_start(hs16d[256:512, :], hs[1792:2048, :])
            for hc in range(HC):
                nc.sync.dma_start(
                    hsTt[hc][3][:], hs16d[:, hc * P : (hc + 1) * P], transpose=True
                )

            psQ_stack = ExitStack()
            psQ = psQ_stack.enter_context(
                tc.tile_pool(name="psQ", bufs=1, space="PSUM")
            )

            def hs_transpose(stg, hcs):
                for hc in hcs:
                    pt = psT.tile([P, 512], BF16, tag="t", bufs=2)
                    for j in range(4):
                        g, jj = divmod(stg * 4 + j, 2)
                        nc.tensor.transpose(
                            pt[:, j * P : (j + 1) * P],
                            hs16[g][:, jj, hc * P : (hc + 1) * P],
                            id16[:],
                        )
                    nc.vector.tensor_copy(hsTt[hc][stg][:], pt[:])

            for stg in range(2):
                hs_transpose(stg, range(HC))

            # ---- work queue machinery ----
            work = deque()

            def pump(n=None):
                if n is None:
                    n = 2 if len(work) > 22 else 1
                for _ in range(n):
                    if not work:
                        return
                    work.popleft()()

            proj_state = {}
            vt_tiles = {}
            psC_holder = {}

            def proj_quarter(dst_chunks, b_sb, w_off, dc, scg, q):
                scs = (2 * scg, 2 * scg + 1)
                key = (w_off, dc, scg)
                if q == 0:
                    proj_state[key] = [
                        psQ.tile([P, QW], F32, tag="ps", bufs=2, name=f"pp{i}")
                        for i in range(2)
                    ]
                pps = proj_state[key]
                for hc in range(2 * q, 2 * q + 2):
                    for i, sc in enumerate(scs):
                        nc.tensor.matmul(
                            pps[i][:],
                            lhsT=w_sb[:, hc, w_off + dc * P : w_off + (dc + 1) * P],
                            rhs=hsTt[hc][sc][:],
                            start=(hc == 0),
                            stop=(hc == HC - 1),
                        )
                if q == 3:
                    for i, sc in enumerate(scs):
                        nc.vector.tensor_scalar_add(
                            out=dst_chunks[sc][:],
                            in0=pps[i][:],
                            scalar1=b_sb[:, dc : dc + 1],
                        )
                    del proj_state[key]

            def proj_group(dst_chunks, b_sb, w_off, dc, scg):
                for q in range(4):
                    proj_quarter(dst_chunks, b_sb, w_off, dc, scg, q)

            def v_dst(dc):
                if dc not in vt_tiles:
                    vt_tiles[dc] = ctxp.tile(
                        [P, S], BF16, tag=f"vt{dc}", bufs=1, name=f"vt{dc}"
                    )
                vt = vt_tiles[dc]
                return [vt[:, sc * QW : (sc + 1) * QW] for sc in range(QC)]

            def v_back(dc, stg):
                vt = vt_tiles[dc]
                pt = psT.tile([P, 512], BF16, tag="t", bufs=2)
                for j in range(4):
                    st = stg * 4 + j
                    nc.tensor.transpose(
                        pt[:, j * P : (j + 1) * P],
                        vt[:, st * P : (st + 1) * P],
                        id16[:],
                    )
                nc.vector.tensor_copy(
                    v_sb[:, stg * 4 : (stg + 1) * 4, 2 * dc : 2 * dc + 2, 0:HD],
                    pt[:].rearrange("p (a h d) -> p a h d", h=2, d=HD),
                )

            def pool_switch():
                psQ_stack.close()
                psC_holder["pool"] = tc.alloc_tile_pool(
                    name="psC", bufs=1, space="PSUM"
                )

            # ---- attention emitters ----
            def scores_emit(pair, qcg):
                pts = {0: [], 1: []}
                q0, q1 = 2 * qcg, 2 * qcg + 1
                for kt in range(ST):
                    sc, kk = divmod(kt, 4)
                    for hh, rows, tp in (
                        (0, slice(0, 64), (0, 0)),
                        (1, slice(64, 128), (64, 0)),
                    ):
                        sps = psS.tile([P, 2 * QW], F32, tag=f"s{hh}", bufs=1)
                        for j, qq in ((0, q0), (1, q1)):
                            nc.tensor.matmul(
                                sps[:, j * QW : (j + 1) * QW],
                                lhsT=kTc[pair][sc][rows, kk * P : (kk + 1) * P],
                                rhs=qTc[pair][qq][rows, :],
                                start=True,
                                stop=True,
                                tile_position=tp,
                            )
                        pt = probsp.tile(
                            [P, 2, QW], BF16, tag=f"p{hh}", bufs=17,
                            name=f"pt{hh}_{kt}",
                        )
                        if plain_mask:
                            nc.scalar.activation(
                                pt[:],
                                sps[:].rearrange("p (a b) -> p a b", b=QW),
                                EXP,
                                scale=0.125,
                            )
                        else:
                            nc.scalar.activation(
                                pt[:],
                                sps[:].rearrange("p (a b) -> p a b", b=QW),
                                EXP,
                                bias=mask_sb[:, kt : kt + 1],
                                scale=0.125,
                            )
                        pts[hh].append(pt)
                    pump()
                return pts

            def ctx_pieces(pair, qcg, pts):
                pieces = []
                for hh in range(2):
                    h = 2 * pair + hh
                    pcs = [None, None]

                    def make_accum(kq, hh=hh, h=h, pcs=pcs):
                        def accum():
                            if kq == 0:
                                psC = psC_holder["pool"]
                                for j in range(2):
                                    pcs[j] = psC.tile(
                                        [P, QW], F32, tag="ca", bufs=2,
                                        name=f"pc{hh}{j}",
                                    )
                            for kt in range(4 * kq, 4 * kq + 4):
                                for j in range(2):
                                    nc.tensor.matmul(
                                        pcs[j][0 : HD + 1, :],
                                        lhsT=v_sb[:, kt, h, :],
                                        rhs=pts[hh][kt][:, j],
                                        start=(kt == 0),
                                        stop=(kt == ST - 1),
                                        skip_group_check=True,
                                    )

                        return accum

                    for kq in range(4):
                        pieces.append(make_accum(kq))

                    def make_post(j, hh=hh, h=h, pcs=pcs):
                        def post():
                            qq = 2 * qcg + j
                            ctxs = ctxp.tile([P, QW], F32, tag="ctxs", bufs=2)
                            nc.vector.tensor_copy(
                                ctxs[0 : HD + 1, :], pcs[j][0 : HD + 1, :]
                            )
                            pd = psT.tile([P, QC * (HD + 1)], F32, tag="t", bufs=2)
                            pdv = pd[:].rearrange("p (q e) -> p q e", e=HD + 1)
                            for qt in range(QC):
                                nc.tensor.transpose(
                                    pdv[:, qt],
                                    ctxs[0 : HD + 1, qt * P : (qt + 1) * P],
                                    id32[0 : HD + 1, 0 : HD + 1],
                                )
                            rec = miscp.tile([P, QC], F32, tag="rec")
                            nc.vector.reciprocal(rec[:], pdv[:, :, HD])
                            ot = outp.tile([P, QC, HD], F32, tag="ot")
                            for qt in range(QC):
                                nc.vector.tensor_scalar_mul(
                                    out=ot[:, qt],
                                    in0=pdv[:, qt, 0:HD],
                                    scalar1=rec[:, qt : qt + 1],
                                )
                            nc.sync.dma_start(
                                y[qq * QW : (qq + 1) * QW, h * HD : (h + 1) * HD]
                                .rearrange("(q p) d -> p q d", p=P),
                                ot[:],
                            )

                        return post

                    pieces.append(make_post(0))
                    pieces.append(make_post(1))
                return pieces

            # ---- emission ----
            # critical path first: q/k chunks covering the first key tiles
            proj_group(qTc[0], bq_sb, 0, 0, 0)
            proj_group(kTc[0], bk_sb, GD, 0, 0)
            # everything else rides the work queue (rest of the hs
            # transposes first -- the kT chunks for key tiles 8..15 need
            # them and must land within the first scores stream)
            work.append(lambda: hs_transpose(2, range(0, 4)))
            work.append(lambda: hs_transpose(2, range(4, 8)))
            for args in (
                (kTc[0], bk_sb, GD, 0, 1),
                (qTc[0], bq_sb, 0, 0, 1),
                (qTc[1], bq_sb, 0, 1, 0),
                (qTc[1], bq_sb, 0, 1, 1),
                (kTc[1], bk_sb, GD, 1, 0),
                (kTc[1], bk_sb, GD, 1, 1),
            ):
                for q in range(4):
                    work.append(lambda a=args, q=q: proj_quarter(*a, q))
            for dc in range(2):
                for scg in range(2):
                    for q in range(4):
                        work.append(
                            lambda dc=dc, scg=scg, q=q: proj_quarter(
                                v_dst(dc), bv_sb, 2 * GD, dc, scg, q
                            )
                        )
            for dc in range(2):
                for stg in range(QC):
                    work.append(lambda dc=dc, stg=stg: v_back(dc, stg))
            work.append(pool_switch)

            pts = scores_emit(0, 0)
            prev = (0, 0, pts)
            for pair, qcg in ((0, 1), (1, 0), (1, 1)):
                work.extend(ctx_pieces(prev[0], prev[1], prev[2]))
                pts = scores_emit(pair, qcg)
                prev = (pair, qcg, pts)
            while work:
                pump(4)
            for fn in ctx_pieces(prev[0], prev[1], prev[2]):
                fn()
            if "pool" in psC_holder:
                psC_holder["pool"].release()
    nc.compile()
    return nc


def _make_in_maps(hidden_states, attention_mask, Wq, bq, Wk, bk, Wv, bv):
    min_val = np.finfo(np.float32).min
    in_maps = []
    for c in range(N_CORES):
        b, g = divmod(c, N_CORES // B)
        sl = slice(GD * g, GD * (g + 1))
        small = np.concatenate(
            [
                bq[sl].reshape(2, P).T,
                bk[sl].reshape(2, P).T,
                bv[sl].reshape(2, P).T,
                ((1.0 - attention_mask[b]) * min_val)
                .astype(np.float32)
                .reshape(ST, P)
                .T,
            ],
            axis=1,
        ).astype(np.float32)
        in_maps.append(
            {
                "hs": np.ascontiguousarray(hidden_states[b]),
                "w": np.ascontiguousarray(
                    np.concatenate([Wq[:, sl], Wk[:, sl], Wv[:, sl]], axis=1)
                ),
                "small_t": np.ascontiguousarray(small),
            }
        )
    return in_maps


def kernel(hidden_states, attention_mask, Wq, bq, Wk, bk, Wv, bv):
    hidden_states = np.asarray(hidden_states, dtype=np.float32)
    attention_mask = np.asarray(attention_mask, dtype=np.float32)
    Wq, Wk, Wv = (np.asarray(a, dtype=np.float32) for a in (Wq, Wk, Wv))
    bq, bk, bv = (np.asarray(a, dtype=np.float32) for a in (bq, bk, bv))

    plain = bool(np.all(attention_mask == 1.0))
    key = ("nc", plain)
    if key not in _CACHE:
        _CACHE[key] = _build_nc(plain)
    nc = _CACHE[key]
    _CACHE["nc"] = nc  # most-recent, for test harness reuse

    in_maps = _make_in_maps(hidden_states, attention_mask, Wq, bq, Wk, bk, Wv, bv)
    res = run_bass_kernel_spmd(nc, in_maps, list(range(N_CORES)))
    out = np.empty((B, S, HID), dtype=np.float32)
    for c in range(N_CORES):
        b, g = divmod(c, N_CORES // B)
        out[b, :, GD * g : GD * (g + 1)] = res.results[c]["y"]
    return out
